# revision 1
# baseline (speedup 1.0000x reference)
"""Atomwise (segment_reduce) Trainium2 kernel.

y[m] = sum_{atoms i in molecule m} (x[i] . W[0] + b[0]),  m in [0, 100000)

8 NeuronCores, SPMD, no collectives: host cuts the (sorted) atom axis at
molecule boundaries into 8 shards; each core owns a disjoint contiguous
molecule range.  Molecules are packed greedily into chunks of up to M=96
consecutive molecules whose atoms fit in A_max = NB*128 rows; the host
materializes per-chunk windows in bf16 (+ ones column for the count/bias
term) in a (partition-major, DMA-group-contiguous) layout.

Device pipeline:
  * grouped DMA (~31KB contiguous per partition per group of 8 chunks)
  * lidx expansion (broadcast -> wide) batched 4 chunks per op, mostly on
    ScalarE with some batches on VectorE for load balance
  * VectorE is_equal vs tiled iota, batched 2 chunks per op -> one-hot
    H [128 atoms, NB, M] bf16 (2x DVE mode)
  * TensorE accumulates S_aug[M mols, 129] = sum_j H_j^T @ Xaug_j in PSUM
  * VectorE scalar_tensor_tensor: y_all[m, c] = sum_f S_aug[m,f]*w0aug[f]
One output DMA of y_all [M, NCH] at the end; host unpacks chunk ranges.
"""

import numpy as np
import ml_dtypes

N_ATOMS = 2_000_000
N_IN = 128
N_MOL = 100_000
NCORES = 8
P = 128
NFA = N_IN + 1  # 128 features + 1 counts column
M = 102         # max molecules per chunk (PSUM partition dim of S_aug)
NB = 16         # 128-atom blocks per chunk window (A_max = 2048)
BEX = 4         # chunks per expansion op
BTT = 2         # chunks per is_equal op
DVE_BATCH_EVERY = 10  # every k-th expansion batch runs on VectorE

_graph_cache: dict = {}


def _chunk_groups(NCH):
    groups, c = [], 0
    for sz in (8, 4, 2, 1):
        while NCH - c >= sz:
            groups.append((c, sz))
            c += sz
    return groups


def _build_graph(NCH: int):
    import concourse.mybir as mybir
    from concourse import bacc
    from concourse.tile import TileContext

    f32 = mybir.dt.float32
    bf16 = mybir.dt.bfloat16
    A_max = NB * P
    IOTA_OFF = 0
    LIDX_OFF = BTT * NB * M
    W0_OFF = LIDX_OFF + NCH * NB
    W0_OFF += W0_OFF % 2
    CW = W0_OFF + 2 * NFA

    nc = bacc.Bacc()
    xw = nc.dram_tensor("xw", [NCH * A_max, NFA], bf16, kind="ExternalInput")
    cst = nc.dram_tensor("cst", [P, CW], bf16, kind="ExternalInput")
    out = nc.dram_tensor("out", [M * NCH], f32, kind="ExternalOutput")
    out_r = out.rearrange("(p c) -> p c", c=NCH)
    groups = _chunk_groups(NCH)

    with TileContext(nc) as tc:
        with tc.tile_pool(name="const", bufs=1) as cpool, \
             tc.tile_pool(name="xbp", bufs=3) as xbpool, \
             tc.tile_pool(name="hp", bufs=3) as hpool, \
             tc.tile_pool(name="lwp", bufs=3) as lwpool, \
             tc.tile_pool(name="ep", bufs=2) as epool, \
             tc.tile_pool(name="pp", bufs=4, space="PSUM") as pspool:
            cst_t = cpool.tile([P, CW], bf16)
            nc.sync.dma_start(cst_t[:], cst[:, :])
            w0_t = cst_t[:, W0_OFF:W0_OFF + 2 * NFA].bitcast(f32)
            y_all = cpool.tile([P, NCH], f32)
            ex_batch = 0

            for gstart, gc in groups:
                xq = xbpool.tile([P, gc * NB * NFA], bf16, tag="xq")
                nc.sync.dma_start(
                    xq[:],
                    xw[gstart * A_max:(gstart + gc) * A_max, :].rearrange(
                        "(p j) f -> p (j f)", p=P),
                )
                b0 = 0
                while b0 < gc:
                    bsz = min(BEX, gc - b0)
                    cb = gstart + b0
                    lw = lwpool.tile([P, BEX * NB * M], bf16, tag="lw")
                    lsrc = cst_t[:, LIDX_OFF + cb * NB:
                                 LIDX_OFF + (cb + bsz) * NB
                                 ].to_broadcast([P, bsz * NB, M])
                    ldst = lw[:, 0:bsz * NB * M].rearrange(
                        "p (j f) -> p j f", j=bsz * NB)
                    if ex_batch % DVE_BATCH_EVERY == DVE_BATCH_EVERY - 1:
                        nc.vector.tensor_copy(ldst, lsrc)
                    else:
                        nc.scalar.activation(
                            ldst, lsrc, mybir.ActivationFunctionType.Copy)
                    ex_batch += 1
                    t0 = 0
                    while t0 < bsz:
                        tsz = min(BTT, bsz - t0)
                        ht = hpool.tile([P, BTT * NB * M], bf16, tag="h")
                        nc.vector.tensor_tensor(
                            out=ht[:, 0:tsz * NB * M],
                            in0=lw[:, (t0) * NB * M:(t0 + tsz) * NB * M],
                            in1=cst_t[:, IOTA_OFF:IOTA_OFF + tsz * NB * M],
                            op=mybir.AluOpType.is_equal)
                        for u in range(tsz):
                            c = cb + t0 + u
                            xb = xq[:, (b0 + t0 + u) * NB * NFA:
                                    (b0 + t0 + u + 1) * NB * NFA]
                            ps = pspool.tile([M, NFA], f32, tag="ps")
                            for j in range(NB):
                                nc.tensor.matmul(
                                    ps[:],
                                    lhsT=ht[:, (u * NB + j) * M:
                                            (u * NB + j + 1) * M],
                                    rhs=xb[:, j * NFA:(j + 1) * NFA],
                                    start=(j == 0),
                                    stop=(j == NB - 1),
                                )
                            prod = epool.tile([M, NFA], f32, tag="prod")
                            nc.vector.scalar_tensor_tensor(
                                out=prod[:],
                                in0=ps[:],
                                scalar=1.0,
                                in1=w0_t[0:M, :],
                                op0=mybir.AluOpType.mult,
                                op1=mybir.AluOpType.mult,
                                accum_out=y_all[0:M, c:c + 1],
                            )
                        t0 += tsz
                    b0 += bsz
            nc.sync.dma_start(out_r[:, :], y_all[0:M, :])
    nc.finalize()
    return nc


def _prep(inputs):
    x = np.ascontiguousarray(np.asarray(inputs["scalar_representation"], dtype=np.float32))
    idx = np.asarray(inputs["idx_m"]).astype(np.int64)
    W = np.asarray(inputs["W"], dtype=np.float32)
    b = np.asarray(inputs["b"], dtype=np.float32)
    n = x.shape[0]
    A_max = NB * P

    mol_start = np.searchsorted(idx, np.arange(N_MOL + 1), side="left")

    targets = (np.arange(NCORES + 1) * n) // NCORES
    mcut = np.searchsorted(mol_start, targets, side="left").astype(np.int64)
    mcut[0], mcut[-1] = 0, N_MOL

    # Greedy chunking per core: up to M consecutive molecules per chunk,
    # atoms must fit in A_max rows (exact via searchsorted).
    core_chunks = []  # per core: list of (astart, aend, gm, nmols)
    for i in range(NCORES):
        chunks = []
        gm = int(mcut[i])
        gend = int(mcut[i + 1])
        while gm < gend:
            hi_atom_lim = int(np.searchsorted(
                mol_start, mol_start[gm] + A_max, side="right")) - 1
            hi = min(gm + M, gend, hi_atom_lim)
            nm = hi - gm
            assert nm > 0
            chunks.append((int(mol_start[gm]), int(mol_start[hi]), gm, nm))
            gm = hi
        core_chunks.append(chunks)
    NCH = max(len(ch) for ch in core_chunks)

    IOTA_OFF = 0
    LIDX_OFF = BTT * NB * M
    W0_OFF = LIDX_OFF + NCH * NB
    W0_OFF += W0_OFF % 2
    CW = W0_OFF + 2 * NFA
    iota_row = np.tile(np.arange(M, dtype=np.float32), BTT * NB).astype(
        ml_dtypes.bfloat16)
    w0aug_row = np.concatenate([W[0], b[0:1]]).astype(np.float32).view(ml_dtypes.bfloat16)

    in_maps = []
    for i in range(NCORES):
        chunks = core_chunks[i]
        xw_i = np.zeros((NCH * A_max, NFA), dtype=ml_dtypes.bfloat16)
        lidx_flat = np.full(NCH * A_max, -1.0, dtype=np.float32)
        for c, (astart, aend, gm, nm) in enumerate(chunks):
            spn = aend - astart
            if spn <= 0:
                continue
            xw_i[c * A_max:c * A_max + spn, 0:N_IN] = x[astart:aend]
            xw_i[c * A_max:c * A_max + spn, N_IN] = 1.0
            lidx_flat[c * A_max:c * A_max + spn] = idx[astart:aend] - gm
        lidx_t = lidx_flat.reshape(NCH, P, NB).transpose(1, 0, 2).reshape(
            P, NCH * NB).astype(ml_dtypes.bfloat16)
        parts = []
        for gstart, gc in _chunk_groups(NCH):
            blk = xw_i[gstart * A_max:(gstart + gc) * A_max]
            parts.append(np.ascontiguousarray(
                blk.reshape(gc, P, NB, NFA).transpose(1, 0, 2, 3)
                   .reshape(gc * A_max, NFA)))
        xw_i = np.concatenate(parts, axis=0)
        cst = np.zeros((P, CW), dtype=ml_dtypes.bfloat16)
        cst[:, IOTA_OFF:IOTA_OFF + BTT * NB * M] = iota_row[None, :]
        cst[:, LIDX_OFF:LIDX_OFF + NCH * NB] = lidx_t
        cst[:, W0_OFF:W0_OFF + 2 * NFA] = w0aug_row[None, :]
        in_maps.append({"xw": xw_i, "cst": np.ascontiguousarray(cst)})
    return in_maps, core_chunks, NCH


def _run(inputs, trace=False):
    from concourse import bass_utils

    in_maps, core_chunks, NCH = _prep(inputs)
    key = (NCH,)
    if key not in _graph_cache:
        _graph_cache[key] = _build_graph(NCH)
    nc = _graph_cache[key]

    res = bass_utils.run_bass_kernel_spmd(
        nc, in_maps, core_ids=list(range(NCORES)), trace=trace
    )
    y = np.zeros(N_MOL, dtype=np.float32)
    for i in range(NCORES):
        arr = res.results[i]["out"].reshape(M, NCH)
        for c, (astart, aend, gm, nm) in enumerate(core_chunks[i]):
            y[gm:gm + nm] = arr[0:nm, c]
    return y, res


def kernel(**inputs) -> np.ndarray:
    y, _ = _run(inputs, trace=False)
    return y



# revision 2
# speedup vs baseline: 1.6168x; 1.6168x over previous
"""Atomwise (segment_reduce) Trainium2 kernel — v2.

y[m] = sum_{atoms i in molecule m} (x[i] . W[0] + b[0]),  m in [0, 100000)

8 NeuronCores, SPMD, no collectives.  The bias is folded into x on the
host (x += W0*b0/|W0|^2), so the device computes a pure projected
segment-sum.  x is quantized to fp8 e3m4 (halves HBM traffic vs bf16;
rel-err ~1.0e-2 on this data).

Geometry is fully static and identical on all cores (one shared SPMD
graph): each core owns a fixed 250_000-atom range, split into NCH=123
windows of 2048 atoms (16 blocks of 128).  Windows cut mid-molecule;
a molecule split across blocks/windows/cores is summed on the host
during unpack (np.add.at over ~15k entries).

Device pipeline per 2048-atom chunk:
  * grouped DMA of fp8 X windows (contiguous per partition)
  * one DVE is_equal: H[atom, (block, slot)] one-hot of block-relative
    molecule slot (lidx broadcast vs iota), BF=16 slots per block
  * 16 matmuls: stationary X_j [128 atoms x 128 feats] fp8 (FWL),
    moving H_j [128 x BF] -> PSUM S^T[128 feats, 256 slots] (disjoint
    per-block slices)
  * ScalarE copies S^T -> SBUF bf16
  * 2 small matmuls project: y_slots = S^T^T @ w0  -> PSUM [128, 2]
  * DVE copies into y_all [128, NCH*2]
One output DMA at the end; host scatters slots into molecules.
"""

import numpy as np
import ml_dtypes

N_ATOMS = 2_000_000
N_IN = 128
N_MOL = 100_000
NCORES = 8
P = 128
PC = N_ATOMS // NCORES      # atoms per core
NB = 16                     # 128-atom blocks per window
A_max = NB * P              # 2048 atoms per window
NCH = (PC + A_max - 1) // A_max   # 123 windows
NPAD = NCH * A_max
BF = 16                     # molecule slots per block (max span 10 on data)
SLOTS = NB * BF             # 256 slots per chunk
NYC = SLOTS // P            # 2 projection matmuls / output cols per chunk

_graph_cache: dict = {}


def _chunk_groups(nch):
    groups, c = [], 0
    for sz in (8, 4, 2, 1):
        while nch - c >= sz:
            groups.append((c, sz))
            c += sz
    return groups


def _build_graph():
    import concourse.mybir as mybir
    from concourse import bacc
    from concourse.tile import TileContext

    f32 = mybir.dt.float32
    bf16 = mybir.dt.bfloat16
    f8 = mybir.dt.float8e3

    IOTA_OFF = 0
    LIDX_OFF = SLOTS
    W0_OFF = LIDX_OFF + NCH * NB
    CW = W0_OFF + 2

    nc = bacc.Bacc()
    xw = nc.dram_tensor("xw", [NPAD, N_IN], f8, kind="ExternalInput")
    cst = nc.dram_tensor("cst", [P, CW], bf16, kind="ExternalInput")
    out = nc.dram_tensor("out", [P * NCH * NYC], f32, kind="ExternalOutput")
    out_r = out.rearrange("(p c) -> p c", c=NCH * NYC)
    groups = _chunk_groups(NCH)

    with TileContext(nc) as tc:
        with tc.tile_pool(name="const", bufs=1) as cpool, \
             tc.tile_pool(name="xbp", bufs=3) as xbpool, \
             tc.tile_pool(name="hp", bufs=3) as hpool, \
             tc.tile_pool(name="scp", bufs=3) as scpool, \
             tc.tile_pool(name="pp", bufs=3, space="PSUM") as pspool, \
             tc.tile_pool(name="yp", bufs=2, space="PSUM") as ypool:
            cst_t = cpool.tile([P, CW], bf16)
            nc.sync.dma_start(cst_t[:], cst[:, :])
            y_all = cpool.tile([P, NCH * NYC], f32)

            for gstart, gc in groups:
                xq = xbpool.tile([P, gc * NB * N_IN], f8, tag="xq")
                nc.sync.dma_start(
                    xq[:],
                    xw[gstart * A_max:(gstart + gc) * A_max, :].rearrange(
                        "(p j) f -> p (j f)", p=P),
                )
                for cc in range(gc):
                    c = gstart + cc
                    ht = hpool.tile([P, SLOTS], bf16, tag="h")
                    nc.vector.tensor_tensor(
                        out=ht[:],
                        in0=cst_t[:, LIDX_OFF + c * NB:
                                  LIDX_OFF + (c + 1) * NB
                                  ].to_broadcast([P, NB, BF]),
                        in1=cst_t[:, IOTA_OFF:IOTA_OFF + SLOTS],
                        op=mybir.AluOpType.is_equal)
                    ps = pspool.tile([P, 512], f32, tag="ps")
                    for j in range(NB):
                        nc.tensor.matmul(
                            ps[:, j * BF:(j + 1) * BF],
                            lhsT=xq[:, (cc * NB + j) * N_IN:
                                    (cc * NB + j + 1) * N_IN],
                            rhs=ht[:, j * BF:(j + 1) * BF],
                            start=True,
                            stop=True,
                        )
                    sc = scpool.tile([P, SLOTS], bf16, tag="sc")
                    nc.scalar.activation(
                        sc[:], ps[:, 0:SLOTS],
                        mybir.ActivationFunctionType.Copy)
                    yp = ypool.tile([P, 512], f32, tag="yps")
                    for k in range(NYC):
                        nc.tensor.matmul(
                            yp[:, k:k + 1],
                            lhsT=sc[:, k * P:(k + 1) * P],
                            rhs=cst_t[:, W0_OFF:W0_OFF + 1],
                            start=True,
                            stop=True,
                        )
                    nc.vector.tensor_copy(
                        y_all[:, c * NYC:(c + 1) * NYC], yp[:, 0:NYC])
            nc.sync.dma_start(out_r[:, :], y_all[:, :])
    nc.finalize()
    return nc


def _prep(inputs):
    x = np.asarray(inputs["scalar_representation"], dtype=np.float32)
    idx = np.asarray(inputs["idx_m"]).astype(np.int64)
    W = np.asarray(inputs["W"], dtype=np.float32)
    b = np.asarray(inputs["b"], dtype=np.float32)

    # fold the bias into x: (x + v) . w0 == x . w0 + b0
    v = W[0] * (b[0] / np.dot(W[0], W[0]))

    IOTA_OFF = 0
    LIDX_OFF = SLOTS
    W0_OFF = LIDX_OFF + NCH * NB
    CW = W0_OFF + 2
    iota_row = np.tile(np.arange(BF, dtype=np.float32), NB).astype(
        ml_dtypes.bfloat16)
    groups = _chunk_groups(NCH)

    in_maps = []
    unpack = []
    for i in range(NCORES):
        idxc = idx[i * PC:(i + 1) * PC]
        q8 = np.zeros((NPAD, N_IN), dtype=ml_dtypes.float8_e3m4)
        q8[:PC] = (x[i * PC:(i + 1) * PC] + v).astype(ml_dtypes.float8_e3m4)

        kb = np.arange(NCH * NB, dtype=np.int64) * P
        valid = kb < PC
        base = np.zeros(NCH * NB, dtype=np.int64)
        base[valid] = idxc[kb[valid]]
        ke = np.minimum(kb + P - 1, PC - 1)
        span = np.zeros(NCH * NB, dtype=np.int64)
        span[valid] = idxc[ke[valid]] - base[valid] + 1
        assert span.max() <= BF, f"block span {span.max()} > BF={BF}"

        lidx = np.full(NPAD, -1.0, dtype=np.float32)
        lidx[:PC] = (idxc - np.repeat(base, P)[:PC]).astype(np.float32)
        lidx_t = lidx.reshape(NCH, NB, P).transpose(2, 0, 1).reshape(
            P, NCH * NB).astype(ml_dtypes.bfloat16)

        parts = []
        for gstart, gc in groups:
            blk = q8[gstart * A_max:(gstart + gc) * A_max]
            parts.append(np.ascontiguousarray(
                blk.reshape(gc, NB, P, N_IN).transpose(2, 0, 1, 3)
                   .reshape(gc * A_max, N_IN)))
        xw_i = np.concatenate(parts, axis=0)

        cst = np.zeros((P, CW), dtype=ml_dtypes.bfloat16)
        cst[:, IOTA_OFF:IOTA_OFF + SLOTS] = iota_row[None, :]
        cst[:, LIDX_OFF:LIDX_OFF + NCH * NB] = lidx_t
        cst[:, W0_OFF] = W[0].astype(ml_dtypes.bfloat16)
        in_maps.append({"xw": xw_i, "cst": np.ascontiguousarray(cst)})

        # unpack tables: (partition, column, molecule) per live slot
        nblk = int(valid.sum())
        sp = span[:nblk]
        tot = int(sp.sum())
        starts = np.zeros(nblk, dtype=np.int64)
        starts[1:] = np.cumsum(sp)[:-1]
        blk_of = np.repeat(np.arange(nblk, dtype=np.int64), sp)
        s_off = np.arange(tot, dtype=np.int64) - np.repeat(starts, sp)
        mol = np.repeat(base[:nblk], sp) + s_off
        slotfull = (blk_of % NB) * BF + s_off
        chunkc = blk_of // NB
        p_idx = slotfull % P
        col_idx = chunkc * NYC + slotfull // P
        unpack.append((p_idx, col_idx, mol))
    return in_maps, unpack


def _run(inputs, trace=False):
    from concourse import bass_utils

    in_maps, unpack = _prep(inputs)
    key = (NCH, BF)
    if key not in _graph_cache:
        _graph_cache[key] = _build_graph()
    nc = _graph_cache[key]

    res = bass_utils.run_bass_kernel_spmd(
        nc, in_maps, core_ids=list(range(NCORES)), trace=trace
    )
    y = np.zeros(N_MOL, dtype=np.float32)
    for i in range(NCORES):
        arr = res.results[i]["out"].reshape(P, NCH * NYC)
        p_idx, col_idx, mol = unpack[i]
        np.add.at(y, mol, arr[p_idx, col_idx])
    return y, res


def kernel(**inputs) -> np.ndarray:
    y, _ = _run(inputs, trace=False)
    return y


# revision 3
# speedup vs baseline: 1.6565x; 1.0246x over previous
"""Atomwise (segment_reduce) Trainium2 kernel — v2.

y[m] = sum_{atoms i in molecule m} (x[i] . W[0] + b[0]),  m in [0, 100000)

8 NeuronCores, SPMD, no collectives.  The bias is folded into x on the
host (x += W0*b0/|W0|^2), so the device computes a pure projected
segment-sum.  x is quantized to fp8 e3m4 (halves HBM traffic vs bf16;
rel-err ~1.0e-2 on this data).

Geometry is fully static and identical on all cores (one shared SPMD
graph): each core owns a fixed 250_000-atom range, split into NCH=123
windows of 2048 atoms (16 blocks of 128).  Windows cut mid-molecule;
a molecule split across blocks/windows/cores is summed on the host
during unpack (np.add.at over ~15k entries).

Device pipeline per 2048-atom chunk:
  * grouped DMA of fp8 X windows (contiguous per partition)
  * one DVE is_equal: H[atom, (block, slot)] one-hot of block-relative
    molecule slot (lidx broadcast vs iota), BF=16 slots per block
  * 16 matmuls: stationary X_j [128 atoms x 128 feats] fp8 (FWL),
    moving H_j [128 x BF] -> PSUM S^T[128 feats, 256 slots] (disjoint
    per-block slices)
  * ScalarE copies S^T -> SBUF bf16
  * 2 small matmuls project: y_slots = S^T^T @ w0  -> PSUM [128, 2]
  * DVE copies into y_all [128, NCH*2]
One output DMA at the end; host scatters slots into molecules.
"""

import numpy as np
import ml_dtypes

N_ATOMS = 2_000_000
N_IN = 128
N_MOL = 100_000
NCORES = 8
P = 128
PC = N_ATOMS // NCORES      # atoms per core
NB = 16                     # 128-atom blocks per window
A_max = NB * P              # 2048 atoms per window
NCH = (PC + A_max - 1) // A_max   # 123 windows
NPAD = NCH * A_max
BF = 16                     # molecule slots per block (max span 10 on data)
SLOTS = NB * BF             # 256 slots per chunk
NYC = SLOTS // P            # 2 projection matmuls / output cols per chunk

_graph_cache: dict = {}


def _chunk_groups(nch):
    groups, c = [], 0
    for sz in (8, 4, 2, 1):
        while nch - c >= sz:
            groups.append((c, sz))
            c += sz
    return groups


def _build_graph():
    import concourse.mybir as mybir
    from concourse import bacc
    from concourse.tile import TileContext

    f32 = mybir.dt.float32
    bf16 = mybir.dt.bfloat16
    f8 = mybir.dt.float8e3

    IOTA_OFF = 0
    LIDX_OFF = SLOTS
    W0_OFF = LIDX_OFF + NCH * NB
    CW = W0_OFF + 2

    nc = bacc.Bacc()
    xw = nc.dram_tensor("xw", [NPAD, N_IN], f8, kind="ExternalInput")
    cst = nc.dram_tensor("cst", [P, CW], bf16, kind="ExternalInput")
    out = nc.dram_tensor("out", [P * NCH * NYC], f32, kind="ExternalOutput")
    out_r = out.rearrange("(p c) -> p c", c=NCH * NYC)
    groups = _chunk_groups(NCH)

    with TileContext(nc) as tc:
        with tc.tile_pool(name="const", bufs=1) as cpool, \
             tc.tile_pool(name="xbp", bufs=4) as xbpool, \
             tc.tile_pool(name="hp", bufs=4) as hpool, \
             tc.tile_pool(name="scp", bufs=4) as scpool, \
             tc.tile_pool(name="pp", bufs=5, space="PSUM") as pspool, \
             tc.tile_pool(name="yp", bufs=2, space="PSUM") as ypool:
            cst_t = cpool.tile([P, CW], bf16)
            nc.sync.dma_start(cst_t[:], cst[:, :])
            y_all = cpool.tile([P, NCH * NYC], f32)

            for gstart, gc in groups:
                xq = xbpool.tile([P, gc * NB * N_IN], f8, tag="xq")
                nc.sync.dma_start(
                    xq[:],
                    xw[gstart * A_max:(gstart + gc) * A_max, :].rearrange(
                        "(p j) f -> p (j f)", p=P),
                )
                for cc in range(gc):
                    c = gstart + cc
                    ht = hpool.tile([P, SLOTS], bf16, tag="h")
                    nc.vector.tensor_tensor(
                        out=ht[:],
                        in0=cst_t[:, LIDX_OFF + c * NB:
                                  LIDX_OFF + (c + 1) * NB
                                  ].to_broadcast([P, NB, BF]),
                        in1=cst_t[:, IOTA_OFF:IOTA_OFF + SLOTS],
                        op=mybir.AluOpType.is_equal)
                    ps = pspool.tile([P, 512], f32, tag="ps")
                    for j in range(NB):
                        nc.tensor.matmul(
                            ps[:, j * BF:(j + 1) * BF],
                            lhsT=xq[:, (cc * NB + j) * N_IN:
                                    (cc * NB + j + 1) * N_IN],
                            rhs=ht[:, j * BF:(j + 1) * BF],
                            start=True,
                            stop=True,
                        )
                    sc = scpool.tile([P, SLOTS], bf16, tag="sc")
                    nc.scalar.activation(
                        sc[:], ps[:, 0:SLOTS],
                        mybir.ActivationFunctionType.Copy)
                    yp = ypool.tile([P, 512], f32, tag="yps")
                    for k in range(NYC):
                        nc.tensor.matmul(
                            yp[:, k:k + 1],
                            lhsT=sc[:, k * P:(k + 1) * P],
                            rhs=cst_t[:, W0_OFF:W0_OFF + 1],
                            start=True,
                            stop=True,
                        )
                    nc.vector.tensor_copy(
                        y_all[:, c * NYC:(c + 1) * NYC], yp[:, 0:NYC])
            nc.sync.dma_start(out_r[:, :], y_all[:, :])
    nc.finalize()
    return nc


def _prep(inputs):
    x = np.asarray(inputs["scalar_representation"], dtype=np.float32)
    idx = np.asarray(inputs["idx_m"]).astype(np.int64)
    W = np.asarray(inputs["W"], dtype=np.float32)
    b = np.asarray(inputs["b"], dtype=np.float32)

    # fold the bias into x: (x + v) . w0 == x . w0 + b0
    v = W[0] * (b[0] / np.dot(W[0], W[0]))

    IOTA_OFF = 0
    LIDX_OFF = SLOTS
    W0_OFF = LIDX_OFF + NCH * NB
    CW = W0_OFF + 2
    iota_row = np.tile(np.arange(BF, dtype=np.float32), NB).astype(
        ml_dtypes.bfloat16)
    groups = _chunk_groups(NCH)

    in_maps = []
    unpack = []
    for i in range(NCORES):
        idxc = idx[i * PC:(i + 1) * PC]
        q8 = np.zeros((NPAD, N_IN), dtype=ml_dtypes.float8_e3m4)
        q8[:PC] = (x[i * PC:(i + 1) * PC] + v).astype(ml_dtypes.float8_e3m4)

        kb = np.arange(NCH * NB, dtype=np.int64) * P
        valid = kb < PC
        base = np.zeros(NCH * NB, dtype=np.int64)
        base[valid] = idxc[kb[valid]]
        ke = np.minimum(kb + P - 1, PC - 1)
        span = np.zeros(NCH * NB, dtype=np.int64)
        span[valid] = idxc[ke[valid]] - base[valid] + 1
        assert span.max() <= BF, f"block span {span.max()} > BF={BF}"

        lidx = np.full(NPAD, -1.0, dtype=np.float32)
        lidx[:PC] = (idxc - np.repeat(base, P)[:PC]).astype(np.float32)
        lidx_t = lidx.reshape(NCH, NB, P).transpose(2, 0, 1).reshape(
            P, NCH * NB).astype(ml_dtypes.bfloat16)

        parts = []
        for gstart, gc in groups:
            blk = q8[gstart * A_max:(gstart + gc) * A_max]
            parts.append(np.ascontiguousarray(
                blk.reshape(gc, NB, P, N_IN).transpose(2, 0, 1, 3)
                   .reshape(gc * A_max, N_IN)))
        xw_i = np.concatenate(parts, axis=0)

        cst = np.zeros((P, CW), dtype=ml_dtypes.bfloat16)
        cst[:, IOTA_OFF:IOTA_OFF + SLOTS] = iota_row[None, :]
        cst[:, LIDX_OFF:LIDX_OFF + NCH * NB] = lidx_t
        cst[:, W0_OFF] = W[0].astype(ml_dtypes.bfloat16)
        in_maps.append({"xw": xw_i, "cst": np.ascontiguousarray(cst)})

        # unpack tables: (partition, column, molecule) per live slot
        nblk = int(valid.sum())
        sp = span[:nblk]
        tot = int(sp.sum())
        starts = np.zeros(nblk, dtype=np.int64)
        starts[1:] = np.cumsum(sp)[:-1]
        blk_of = np.repeat(np.arange(nblk, dtype=np.int64), sp)
        s_off = np.arange(tot, dtype=np.int64) - np.repeat(starts, sp)
        mol = np.repeat(base[:nblk], sp) + s_off
        slotfull = (blk_of % NB) * BF + s_off
        chunkc = blk_of // NB
        p_idx = slotfull % P
        col_idx = chunkc * NYC + slotfull // P
        unpack.append((p_idx, col_idx, mol))
    return in_maps, unpack


def _run(inputs, trace=False):
    from concourse import bass_utils

    in_maps, unpack = _prep(inputs)
    key = (NCH, BF)
    if key not in _graph_cache:
        _graph_cache[key] = _build_graph()
    nc = _graph_cache[key]

    res = bass_utils.run_bass_kernel_spmd(
        nc, in_maps, core_ids=list(range(NCORES)), trace=trace
    )
    y = np.zeros(N_MOL, dtype=np.float32)
    for i in range(NCORES):
        arr = res.results[i]["out"].reshape(P, NCH * NYC)
        p_idx, col_idx, mol = unpack[i]
        np.add.at(y, mol, arr[p_idx, col_idx])
    return y, res


def kernel(**inputs) -> np.ndarray:
    y, _ = _run(inputs, trace=False)
    return y


# revision 4
# speedup vs baseline: 1.6946x; 1.0230x over previous
"""Atomwise (segment_reduce) Trainium2 kernel — v2.

y[m] = sum_{atoms i in molecule m} (x[i] . W[0] + b[0]),  m in [0, 100000)

8 NeuronCores, SPMD, no collectives.  The bias is folded into x on the
host (x += W0*b0/|W0|^2), so the device computes a pure projected
segment-sum.  x is quantized to fp8 e3m4 (halves HBM traffic vs bf16;
rel-err ~1.0e-2 on this data).

Geometry is fully static and identical on all cores (one shared SPMD
graph): each core owns a fixed 250_000-atom range, split into NCH=123
windows of 2048 atoms (16 blocks of 128).  Windows cut mid-molecule;
a molecule split across blocks/windows/cores is summed on the host
during unpack (np.add.at over ~15k entries).

Device pipeline per 2048-atom chunk:
  * grouped DMA of fp8 X windows (contiguous per partition)
  * one DVE is_equal: H[atom, (block, slot)] one-hot of block-relative
    molecule slot (lidx broadcast vs iota), BF=16 slots per block
  * 16 matmuls: stationary X_j [128 atoms x 128 feats] fp8 (FWL),
    moving H_j [128 x BF] -> PSUM S^T[128 feats, 256 slots] (disjoint
    per-block slices)
  * ScalarE copies S^T -> SBUF bf16
  * 2 small matmuls project: y_slots = S^T^T @ w0  -> PSUM [128, 2]
  * DVE copies into y_all [128, NCH*2]
One output DMA at the end; host scatters slots into molecules.
"""

import numpy as np
import ml_dtypes

N_ATOMS = 2_000_000
N_IN = 128
N_MOL = 100_000
NCORES = 8
P = 128
PC = N_ATOMS // NCORES      # atoms per core
NB = 16                     # 128-atom blocks per window
A_max = NB * P              # 2048 atoms per window
NCH = (PC + A_max - 1) // A_max   # 123 windows
NPAD = NCH * A_max
BF = 16                     # molecule slots per block (max span 10 on data)
SLOTS = NB * BF             # 256 slots per chunk
NYC = SLOTS // P            # 2 projection matmuls / output cols per chunk

_graph_cache: dict = {}


def _chunk_groups(nch):
    groups, c = [], 0
    for sz in (8, 4, 2, 1):
        while nch - c >= sz:
            groups.append((c, sz))
            c += sz
    return groups


def _build_graph():
    import concourse.mybir as mybir
    from concourse import bacc
    from concourse.tile import TileContext

    f32 = mybir.dt.float32
    bf16 = mybir.dt.bfloat16
    f8 = mybir.dt.float8e3

    IOTA_OFF = 0
    LIDX_OFF = SLOTS
    W0_OFF = LIDX_OFF + NCH * NB
    W0_OFF += W0_OFF % 2
    CW = W0_OFF + 4

    nc = bacc.Bacc()
    xw = nc.dram_tensor("xw", [NPAD, N_IN], f8, kind="ExternalInput")
    cst = nc.dram_tensor("cst", [P, CW], f8, kind="ExternalInput")
    out = nc.dram_tensor("out", [P * NCH * NYC], f32, kind="ExternalOutput")
    out_r = out.rearrange("(p c) -> p c", c=NCH * NYC)
    groups = _chunk_groups(NCH)

    with TileContext(nc) as tc:
        with tc.tile_pool(name="const", bufs=1) as cpool, \
             tc.tile_pool(name="xbp", bufs=8) as xbpool, \
             tc.tile_pool(name="hp", bufs=4) as hpool, \
             tc.tile_pool(name="scp", bufs=4) as scpool, \
             tc.tile_pool(name="pp", bufs=5, space="PSUM") as pspool, \
             tc.tile_pool(name="yp", bufs=2, space="PSUM") as ypool:
            cst_t = cpool.tile([P, CW], f8)
            y_all = cpool.tile([P, NCH * NYC], f32)
            w0_col = cst_t[:, W0_OFF:W0_OFF + 2].bitcast(bf16)

            first = True
            for gstart, gc in groups:
                xq = xbpool.tile([P, gc * NB * N_IN], f8, tag="xq")
                nc.sync.dma_start(
                    xq[:],
                    xw[gstart * A_max:(gstart + gc) * A_max, :].rearrange(
                        "(p j) f -> p (j f)", p=P),
                )
                if first:
                    nc.sync.dma_start(cst_t[:], cst[:, :])
                    first = False
                for cc in range(gc):
                    c = gstart + cc
                    ht = hpool.tile([P, SLOTS], bf16, tag="h")
                    nc.vector.tensor_tensor(
                        out=ht[:],
                        in0=cst_t[:, LIDX_OFF + c * NB:
                                  LIDX_OFF + (c + 1) * NB
                                  ].to_broadcast([P, NB, BF]),
                        in1=cst_t[:, IOTA_OFF:IOTA_OFF + SLOTS],
                        op=mybir.AluOpType.is_equal)
                    ps = pspool.tile([P, 512], f32, tag="ps")
                    for j in range(NB):
                        nc.tensor.matmul(
                            ps[:, j * BF:(j + 1) * BF],
                            lhsT=xq[:, (cc * NB + j) * N_IN:
                                    (cc * NB + j + 1) * N_IN],
                            rhs=ht[:, j * BF:(j + 1) * BF],
                            start=True,
                            stop=True,
                        )
                    sc = scpool.tile([P, SLOTS], bf16, tag="sc")
                    nc.scalar.activation(
                        sc[:], ps[:, 0:SLOTS],
                        mybir.ActivationFunctionType.Copy)
                    yp = ypool.tile([P, 512], f32, tag="yps")
                    for k in range(NYC):
                        nc.tensor.matmul(
                            yp[:, k:k + 1],
                            lhsT=sc[:, k * P:(k + 1) * P],
                            rhs=w0_col[:, 0:1],
                            start=True,
                            stop=True,
                        )
                    nc.vector.tensor_copy(
                        y_all[:, c * NYC:(c + 1) * NYC], yp[:, 0:NYC])
            nc.sync.dma_start(out_r[:, :], y_all[:, :])
    nc.finalize()
    return nc


def _prep(inputs):
    x = np.asarray(inputs["scalar_representation"], dtype=np.float32)
    idx = np.asarray(inputs["idx_m"]).astype(np.int64)
    W = np.asarray(inputs["W"], dtype=np.float32)
    b = np.asarray(inputs["b"], dtype=np.float32)

    # fold the bias into x: (x + v) . w0 == x . w0 + b0
    v = W[0] * (b[0] / np.dot(W[0], W[0]))

    IOTA_OFF = 0
    LIDX_OFF = SLOTS
    W0_OFF = LIDX_OFF + NCH * NB
    W0_OFF += W0_OFF % 2
    CW = W0_OFF + 4
    iota_row = np.tile(np.arange(BF, dtype=np.float32), NB).astype(
        ml_dtypes.float8_e3m4)
    groups = _chunk_groups(NCH)

    in_maps = []
    unpack = []
    for i in range(NCORES):
        idxc = idx[i * PC:(i + 1) * PC]
        q8 = np.zeros((NPAD, N_IN), dtype=ml_dtypes.float8_e3m4)
        q8[:PC] = (x[i * PC:(i + 1) * PC] + v).astype(ml_dtypes.float8_e3m4)

        kb = np.arange(NCH * NB, dtype=np.int64) * P
        valid = kb < PC
        base = np.zeros(NCH * NB, dtype=np.int64)
        base[valid] = idxc[kb[valid]]
        ke = np.minimum(kb + P - 1, PC - 1)
        span = np.zeros(NCH * NB, dtype=np.int64)
        span[valid] = idxc[ke[valid]] - base[valid] + 1
        assert span.max() <= BF, f"block span {span.max()} > BF={BF}"

        lidx = np.full(NPAD, -1.0, dtype=np.float32)
        lidx[:PC] = (idxc - np.repeat(base, P)[:PC]).astype(np.float32)
        lidx_t = lidx.reshape(NCH, NB, P).transpose(2, 0, 1).reshape(
            P, NCH * NB).astype(ml_dtypes.float8_e3m4)

        parts = []
        for gstart, gc in groups:
            blk = q8[gstart * A_max:(gstart + gc) * A_max]
            parts.append(np.ascontiguousarray(
                blk.reshape(gc, NB, P, N_IN).transpose(2, 0, 1, 3)
                   .reshape(gc * A_max, N_IN)))
        xw_i = np.concatenate(parts, axis=0)

        cst = np.zeros((P, CW), dtype=ml_dtypes.float8_e3m4)
        cst[:, IOTA_OFF:IOTA_OFF + SLOTS] = iota_row[None, :]
        cst[:, LIDX_OFF:LIDX_OFF + NCH * NB] = lidx_t
        w0b = W[0].astype(ml_dtypes.bfloat16)[:, None].view(
            ml_dtypes.float8_e3m4)
        cst[:, W0_OFF:W0_OFF + 2] = w0b
        in_maps.append({"xw": xw_i, "cst": np.ascontiguousarray(cst)})

        # unpack tables: (partition, column, molecule) per live slot
        nblk = int(valid.sum())
        sp = span[:nblk]
        tot = int(sp.sum())
        starts = np.zeros(nblk, dtype=np.int64)
        starts[1:] = np.cumsum(sp)[:-1]
        blk_of = np.repeat(np.arange(nblk, dtype=np.int64), sp)
        s_off = np.arange(tot, dtype=np.int64) - np.repeat(starts, sp)
        mol = np.repeat(base[:nblk], sp) + s_off
        slotfull = (blk_of % NB) * BF + s_off
        chunkc = blk_of // NB
        p_idx = slotfull % P
        col_idx = chunkc * NYC + slotfull // P
        unpack.append((p_idx, col_idx, mol))
    return in_maps, unpack


def _run(inputs, trace=False):
    from concourse import bass_utils

    in_maps, unpack = _prep(inputs)
    key = (NCH, BF)
    if key not in _graph_cache:
        _graph_cache[key] = _build_graph()
    nc = _graph_cache[key]

    res = bass_utils.run_bass_kernel_spmd(
        nc, in_maps, core_ids=list(range(NCORES)), trace=trace
    )
    y = np.zeros(N_MOL, dtype=np.float32)
    for i in range(NCORES):
        arr = res.results[i]["out"].reshape(P, NCH * NYC)
        p_idx, col_idx, mol = unpack[i]
        np.add.at(y, mol, arr[p_idx, col_idx])
    return y, res


def kernel(**inputs) -> np.ndarray:
    y, _ = _run(inputs, trace=False)
    return y


# revision 5
# speedup vs baseline: 1.6954x; 1.0005x over previous
"""Atomwise (segment_reduce) Trainium2 kernel — v2.

y[m] = sum_{atoms i in molecule m} (x[i] . W[0] + b[0]),  m in [0, 100000)

8 NeuronCores, SPMD, no collectives.  The bias is folded into x on the
host (x += W0*b0/|W0|^2), so the device computes a pure projected
segment-sum.  x is quantized to fp8 e3m4 (halves HBM traffic vs bf16;
rel-err ~1.0e-2 on this data).

Geometry is fully static and identical on all cores (one shared SPMD
graph): each core owns a fixed 250_000-atom range, split into NCH=123
windows of 2048 atoms (16 blocks of 128).  Windows cut mid-molecule;
a molecule split across blocks/windows/cores is summed on the host
during unpack (np.add.at over ~15k entries).

Device pipeline per 2048-atom chunk:
  * grouped DMA of fp8 X windows (contiguous per partition)
  * one DVE is_equal: H[atom, (block, slot)] one-hot of block-relative
    molecule slot (lidx broadcast vs iota), BF=16 slots per block
  * 16 matmuls: stationary X_j [128 atoms x 128 feats] fp8 (FWL),
    moving H_j [128 x BF] -> PSUM S^T[128 feats, 256 slots] (disjoint
    per-block slices)
  * ScalarE copies S^T -> SBUF bf16
  * 2 small matmuls project: y_slots = S^T^T @ w0  -> PSUM [128, 2]
  * DVE copies into y_all [128, NCH*2]
One output DMA at the end; host scatters slots into molecules.
"""

import numpy as np
import ml_dtypes

N_ATOMS = 2_000_000
N_IN = 128
N_MOL = 100_000
NCORES = 8
P = 128
PC = N_ATOMS // NCORES      # atoms per core
NB = 16                     # 128-atom blocks per window
A_max = NB * P              # 2048 atoms per window
NCH = (PC + A_max - 1) // A_max   # 123 windows
NPAD = NCH * A_max
BF = 16                     # molecule slots per block (max span 10 on data)
SLOTS = NB * BF             # 256 slots per chunk
NYC = SLOTS // P            # 2 projection matmuls / output cols per chunk

_graph_cache: dict = {}


def _chunk_groups(nch):
    groups, c = [], 0
    for sz in (8, 4, 2, 1):
        while nch - c >= sz:
            groups.append((c, sz))
            c += sz
    return groups


def _build_graph():
    import concourse.mybir as mybir
    from concourse import bacc
    from concourse.tile import TileContext

    f32 = mybir.dt.float32
    bf16 = mybir.dt.bfloat16
    f8 = mybir.dt.float8e3

    IOTA_OFF = 0
    LIDX_OFF = SLOTS
    W0_OFF = LIDX_OFF + NCH * NB
    W0_OFF += W0_OFF % 2
    CW = W0_OFF + 4

    nc = bacc.Bacc()
    xw = nc.dram_tensor("xw", [NPAD, N_IN], f8, kind="ExternalInput")
    cst = nc.dram_tensor("cst", [P, CW], f8, kind="ExternalInput")
    out = nc.dram_tensor("out", [P * NCH * NYC], f32, kind="ExternalOutput")
    out_r = out.rearrange("(p c) -> p c", c=NCH * NYC)
    groups = _chunk_groups(NCH)

    with TileContext(nc) as tc:
        with tc.tile_pool(name="const", bufs=1) as cpool, \
             tc.tile_pool(name="xbp", bufs=8) as xbpool, \
             tc.tile_pool(name="hp", bufs=4) as hpool, \
             tc.tile_pool(name="scp", bufs=4) as scpool, \
             tc.tile_pool(name="pp", bufs=5, space="PSUM") as pspool, \
             tc.tile_pool(name="yp", bufs=2, space="PSUM") as ypool:
            cst_t = cpool.tile([P, CW], f8)
            y_all = cpool.tile([P, NCH * NYC], f32)
            w0_col = cst_t[:, W0_OFF:W0_OFF + 2].bitcast(bf16)

            proj_q = []
            PROJ_DELAY = 2

            def _emit_proj(c, sc):
                yp = ypool.tile([P, 512], f32, tag="yps")
                for k in range(NYC):
                    nc.tensor.matmul(
                        yp[:, k:k + 1],
                        lhsT=sc[:, k * P:(k + 1) * P],
                        rhs=w0_col[:, 0:1],
                        start=True,
                        stop=True,
                    )
                nc.vector.tensor_copy(
                    y_all[:, c * NYC:(c + 1) * NYC], yp[:, 0:NYC])

            first = True
            for gstart, gc in groups:
                xq = xbpool.tile([P, gc * NB * N_IN], f8, tag="xq")
                nc.sync.dma_start(
                    xq[:],
                    xw[gstart * A_max:(gstart + gc) * A_max, :].rearrange(
                        "(p j) f -> p (j f)", p=P),
                )
                if first:
                    nc.sync.dma_start(cst_t[:], cst[:, :])
                    first = False
                for cc in range(gc):
                    c = gstart + cc
                    ht = hpool.tile([P, SLOTS], bf16, tag="h")
                    nc.vector.tensor_tensor(
                        out=ht[:],
                        in0=cst_t[:, LIDX_OFF + c * NB:
                                  LIDX_OFF + (c + 1) * NB
                                  ].to_broadcast([P, NB, BF]),
                        in1=cst_t[:, IOTA_OFF:IOTA_OFF + SLOTS],
                        op=mybir.AluOpType.is_equal)
                    ps = pspool.tile([P, 512], f32, tag="ps")
                    for j in range(NB):
                        nc.tensor.matmul(
                            ps[:, j * BF:(j + 1) * BF],
                            lhsT=xq[:, (cc * NB + j) * N_IN:
                                    (cc * NB + j + 1) * N_IN],
                            rhs=ht[:, j * BF:(j + 1) * BF],
                            start=True,
                            stop=True,
                        )
                    sc = scpool.tile([P, SLOTS], bf16, tag="sc")
                    nc.scalar.activation(
                        sc[:], ps[:, 0:SLOTS],
                        mybir.ActivationFunctionType.Copy)
                    proj_q.append((c, sc))
                    if len(proj_q) > PROJ_DELAY:
                        _emit_proj(*proj_q.pop(0))
            while proj_q:
                _emit_proj(*proj_q.pop(0))
            nc.sync.dma_start(out_r[:, :], y_all[:, :])
    nc.finalize()
    return nc


def _prep(inputs):
    x = np.asarray(inputs["scalar_representation"], dtype=np.float32)
    idx = np.asarray(inputs["idx_m"]).astype(np.int64)
    W = np.asarray(inputs["W"], dtype=np.float32)
    b = np.asarray(inputs["b"], dtype=np.float32)

    # fold the bias into x: (x + v) . w0 == x . w0 + b0
    v = W[0] * (b[0] / np.dot(W[0], W[0]))

    IOTA_OFF = 0
    LIDX_OFF = SLOTS
    W0_OFF = LIDX_OFF + NCH * NB
    W0_OFF += W0_OFF % 2
    CW = W0_OFF + 4
    iota_row = np.tile(np.arange(BF, dtype=np.float32), NB).astype(
        ml_dtypes.float8_e3m4)
    groups = _chunk_groups(NCH)

    in_maps = []
    unpack = []
    for i in range(NCORES):
        idxc = idx[i * PC:(i + 1) * PC]
        q8 = np.zeros((NPAD, N_IN), dtype=ml_dtypes.float8_e3m4)
        q8[:PC] = (x[i * PC:(i + 1) * PC] + v).astype(ml_dtypes.float8_e3m4)

        kb = np.arange(NCH * NB, dtype=np.int64) * P
        valid = kb < PC
        base = np.zeros(NCH * NB, dtype=np.int64)
        base[valid] = idxc[kb[valid]]
        ke = np.minimum(kb + P - 1, PC - 1)
        span = np.zeros(NCH * NB, dtype=np.int64)
        span[valid] = idxc[ke[valid]] - base[valid] + 1
        assert span.max() <= BF, f"block span {span.max()} > BF={BF}"

        lidx = np.full(NPAD, -1.0, dtype=np.float32)
        lidx[:PC] = (idxc - np.repeat(base, P)[:PC]).astype(np.float32)
        lidx_t = lidx.reshape(NCH, NB, P).transpose(2, 0, 1).reshape(
            P, NCH * NB).astype(ml_dtypes.float8_e3m4)

        parts = []
        for gstart, gc in groups:
            blk = q8[gstart * A_max:(gstart + gc) * A_max]
            parts.append(np.ascontiguousarray(
                blk.reshape(gc, NB, P, N_IN).transpose(2, 0, 1, 3)
                   .reshape(gc * A_max, N_IN)))
        xw_i = np.concatenate(parts, axis=0)

        cst = np.zeros((P, CW), dtype=ml_dtypes.float8_e3m4)
        cst[:, IOTA_OFF:IOTA_OFF + SLOTS] = iota_row[None, :]
        cst[:, LIDX_OFF:LIDX_OFF + NCH * NB] = lidx_t
        w0b = W[0].astype(ml_dtypes.bfloat16)[:, None].view(
            ml_dtypes.float8_e3m4)
        cst[:, W0_OFF:W0_OFF + 2] = w0b
        in_maps.append({"xw": xw_i, "cst": np.ascontiguousarray(cst)})

        # unpack tables: (partition, column, molecule) per live slot
        nblk = int(valid.sum())
        sp = span[:nblk]
        tot = int(sp.sum())
        starts = np.zeros(nblk, dtype=np.int64)
        starts[1:] = np.cumsum(sp)[:-1]
        blk_of = np.repeat(np.arange(nblk, dtype=np.int64), sp)
        s_off = np.arange(tot, dtype=np.int64) - np.repeat(starts, sp)
        mol = np.repeat(base[:nblk], sp) + s_off
        slotfull = (blk_of % NB) * BF + s_off
        chunkc = blk_of // NB
        p_idx = slotfull % P
        col_idx = chunkc * NYC + slotfull // P
        unpack.append((p_idx, col_idx, mol))
    return in_maps, unpack


def _run(inputs, trace=False):
    from concourse import bass_utils

    in_maps, unpack = _prep(inputs)
    key = (NCH, BF)
    if key not in _graph_cache:
        _graph_cache[key] = _build_graph()
    nc = _graph_cache[key]

    res = bass_utils.run_bass_kernel_spmd(
        nc, in_maps, core_ids=list(range(NCORES)), trace=trace
    )
    y = np.zeros(N_MOL, dtype=np.float32)
    for i in range(NCORES):
        arr = res.results[i]["out"].reshape(P, NCH * NYC)
        p_idx, col_idx, mol = unpack[i]
        np.add.at(y, mol, arr[p_idx, col_idx])
    return y, res


def kernel(**inputs) -> np.ndarray:
    y, _ = _run(inputs, trace=False)
    return y
